# revision 1
# baseline (speedup 1.0000x reference)
"""Bayesian linear layer (reparameterized) on 8 Trainium2 NeuronCores.

y = x @ (mu + exp(log_sigma) * eps_w).T + (bias_mu + exp(bias_log_sigma) * eps_b)

Shapes: x [8192, 4096] f32, weights [16384, 4096] f32, y [8192, 16384] f32.

Strategy (column-parallel / tensor-parallel over out_features):
  - Shard all [OUT, IN] weight tensors and the bias vectors along OUT across
    8 cores (2048 out features per core); replicate x.
  - Host feeds x pre-tiled/bf16 (one contiguous 1 MB DMA per token tile) and
    W-shard transposes [IN, OUT_S] in fp16 so the contraction dim lands on
    SBUF partitions with natural, wide DMA lines.
  - On device: build W.T = mu + exp(ls)*eps in bf16, SBUF-resident, as
    1024-wide column-pair tiles; the output is computed pair-group by
    pair-group so the first pair's token sweep starts as soon as its W is
    built while the second pair's weight inputs stream in behind it.
    Each [128 tok x 512 out] tile accumulates 32 bf16 matmuls in fp32 PSUM;
    bias is added during PSUM eviction on the vector engine; y stores go out
    on the SWDGE queues to keep the load queues latency-clean.
  - Host concatenates the per-core [8192, 2048] f32 outputs along axis 1.
"""

import os
import sys

sys.path.insert(0, "/opt/trn_rl_repo")
os.environ.setdefault("MYCRO_LOCAL_CACHE", "1")

import numpy as np
import ml_dtypes

N_TOK, IN_DIM, OUT_DIM = 8192, 4096, 16384
N_CORES = 8
OUT_S = OUT_DIM // N_CORES  # 2048
P = 128


def build_program(n_tok=N_TOK, in_dim=IN_DIM, out_s=OUT_S, n_cores=N_CORES,
                  chunk=512, xt_bufs=3, out_bufs=4, psum_bufs=8, scratch_bufs=3):
    """Build + compile the single-core Bass program (SPMD across cores)."""
    import concourse.bass as bass
    import concourse.mybir as mybir
    import concourse.tile as tile
    from concourse import bacc
    from contextlib import ExitStack

    fp32 = mybir.dt.float32
    bf16 = mybir.dt.bfloat16
    fp16 = mybir.dt.float16
    Exp = mybir.ActivationFunctionType.Exp
    add = mybir.AluOpType.add

    KT = in_dim // P        # k tiles (contraction)
    MT = n_tok // P         # token tiles
    NO = out_s // 512       # psum-width output chunks
    NCH = out_s // chunk    # elementwise build chunks
    assert in_dim % P == 0 and n_tok % P == 0 and out_s % 512 == 0 and out_s % chunk == 0

    nc = bacc.Bacc("TRN2", target_bir_lowering=False, debug=False,
                   num_devices=n_cores, enable_asserts=False)

    # x pre-tiled on host: xB[m, ki, ko, t] = x[m*128 + t, ko*128 + ki],
    # so each token-tile load is one fully contiguous DMA.
    xB = nc.dram_tensor("xB", [n_tok // P, P, in_dim // P, P], bf16,
                        kind="ExternalInput")
    # fp16 (not bf16): ls ~ -5, and bf16's 8-bit mantissa on ls is a ~1%
    # multiplicative error after exp; fp16's 10 bits keep the whole pipeline
    # at f32-input accuracy (verified numerically) at half the DMA traffic.
    muT = nc.dram_tensor("muT", [in_dim, out_s], fp16, kind="ExternalInput")
    lsT = nc.dram_tensor("lsT", [in_dim, out_s], fp16, kind="ExternalInput")
    epsT = nc.dram_tensor("epsT", [in_dim, out_s], fp16, kind="ExternalInput")
    bmu = nc.dram_tensor("bmu", [out_s], fp32, kind="ExternalInput")
    bls = nc.dram_tensor("bls", [out_s], fp32, kind="ExternalInput")
    beps = nc.dram_tensor("beps", [out_s], fp32, kind="ExternalInput")
    y = nc.dram_tensor("y", [n_tok, out_s], fp32, kind="ExternalOutput")

    with tile.TileContext(nc) as tc, ExitStack() as ctx:
        wt_pool = ctx.enter_context(tc.tile_pool(name="wt", bufs=1))
        const_pool = ctx.enter_context(tc.tile_pool(name="const", bufs=1))
        scratch = ctx.enter_context(tc.tile_pool(name="scratch", bufs=scratch_bufs))
        xt_pool = ctx.enter_context(tc.tile_pool(name="xt", bufs=xt_bufs))
        out_pool = ctx.enter_context(tc.tile_pool(name="out", bufs=out_bufs))
        psum_pool = ctx.enter_context(
            tc.tile_pool(name="psum", bufs=psum_bufs, space="PSUM"))

        def fused_w(dst_ap, ls_src, eps_src, mu_src, sync_engine,
                    dt_ls, dt_em, exp_dt, pre="", bufs=None, width=None):
            # dst = mu + exp(ls) * eps, elementwise over a [P, width] block
            w = chunk if width is None else width
            kw = {} if bufs is None else {"bufs": bufs}
            l = scratch.tile([P, w], dt_ls, tag=pre + "ls", name="ls_t", **kw)
            e = scratch.tile([P, w], dt_em, tag=pre + "eps", name="eps_t", **kw)
            m_ = scratch.tile([P, w], dt_em, tag=pre + "mu", name="mu_t", **kw)
            x_ = scratch.tile([P, w], exp_dt, tag=pre + "exp",
                              name="exp_t", **kw)
            sync_engine.dma_start(out=l[:], in_=ls_src)
            sync_engine.dma_start(out=e[:], in_=eps_src)
            sync_engine.dma_start(out=m_[:], in_=mu_src)
            nc.scalar.activation(x_[:], l[:], Exp)
            nc.vector.tensor_mul(x_[:], x_[:], e[:])
            nc.vector.tensor_tensor(dst_ap, x_[:], m_[:], add)

        # bias_rep[p, o] = bmu[o] + exp(bls[o]) * beps[o]; bf16 is plenty
        # (it is added into the f32 psum at eviction).
        bias_rep = const_pool.tile([P, out_s], bf16, tag="bias_rep",
                                   name="bias_rep")

        def bias_chunk(j):
            sl = slice(j * chunk, (j + 1) * chunk)
            fused_w(bias_rep[:, sl],
                    bls.ap()[sl].partition_broadcast(P),
                    beps.ap()[sl].partition_broadcast(P),
                    bmu.ap()[sl].partition_broadcast(P),
                    nc.gpsimd, fp32, fp32, fp32, pre="b", bufs=1)

        # ---- W build + matmul, grouped by 1024-wide output column pairs ----
        # 1024-wide W tiles give 2 KB-per-partition DMA lines (half the
        # descriptor count of 512-wide ones). Group 0 (first pair) builds
        # first and its full token sweep starts immediately; the second
        # pair's weight inputs stream in behind that compute. W.T stays
        # SBUF-resident in bf16.
        OCW = 1024  # W-tile width; each holds 2 psum-width (512) columns
        assert out_s % OCW == 0
        NP = out_s // OCW
        groups = [[p] for p in range(NP)]

        wt = {}  # (k, p) -> [P, OCW] bf16 tile

        def build_w_chunk(k, p):
            t = wt_pool.tile([P, OCW], bf16, tag=f"wt{k}_{p}",
                             name=f"wt{k}_{p}")
            wt[(k, p)] = t
            rows = slice(k * P, (k + 1) * P)
            sl = slice(p * OCW, (p + 1) * OCW)
            fused_w(t[:],
                    lsT.ap()[rows, sl],
                    epsT.ap()[rows, sl],
                    muT.ap()[rows, sl],
                    nc.sync, fp16, fp16, fp32, width=OCW, bufs=2)

        def load_xt(m):
            xt = xt_pool.tile([P, KT, P], bf16, tag="xt", name="xt")
            nc.sync.dma_start(out=xt[:], in_=xB.ap()[m])
            return xt

        xt_ahead = []  # prefetched token tiles, consumed by the first iters

        for gi, g in enumerate(groups):
            if gi == 0:
                xt_ahead.append(load_xt(0))
                # Warm-up: throwaway matmuls with no W dependency keep the
                # PE dense through the W-build window, so the HAM clock gate
                # opens to 8/8 once and stays (idle >3.4us re-throttles to
                # half clock).
                warm_ps = psum_pool.tile([P, 512], fp32, tag="ps",
                                         name="warm_ps")
                for _ in range(60):
                    nc.tensor.matmul(warm_ps[:, :P], xt_ahead[0][:, 0, :],
                                     xt_ahead[0][:, 1, :],
                                     start=True, stop=True)
                while len(xt_ahead) < min(xt_bufs, MT):
                    xt_ahead.append(load_xt(len(xt_ahead)))
                for k in range(KT):
                    for p in g:
                        build_w_chunk(k, p)
                for oc in range(NO):
                    bias_chunk(oc)
            # Next group's bias/W-build chunks are interleaved into this
            # group's m-loop below so their DMA/ACT/DVE work overlaps matmul
            # compute instead of queueing behind the whole group in program
            # order.
            nxt = groups[gi + 1] if gi + 1 < len(groups) else []
            pending = [(lambda k=k, p=p: build_w_chunk(k, p))
                       for k in range(KT) for p in nxt]
            n_pending = len(pending)
            pending = iter(pending)
            ocs = [p * 2 + j for p in g for j in range(2)]  # 512-wide cols

            def evict(psums, m):
                for oc in ocs:
                    ot = out_pool.tile([P, 512], fp32, tag="ot", name="ot")
                    nc.vector.tensor_tensor(ot[:], psums[oc][:],
                                            bias_rep[:, oc * 512:(oc + 1) * 512],
                                            add)
                    # SWDGE (gpsimd): y stores wait on the eviction, and on
                    # the sync stream that wait head-of-line-blocks the next
                    # x-tile load; stores are latency-insensitive, so keep
                    # them off the load queues entirely.
                    nc.gpsimd.dma_start(
                        out=y.ap()[m * P:(m + 1) * P, oc * 512:(oc + 1) * 512],
                        in_=ot[:])

            def alloc_psums(m):
                return {oc: psum_pool.tile([P, 512], fp32, tag="ps",
                                           name=f"ps{m}_{oc}")
                        for oc in ocs}

            per_iter = -(-n_pending // max(MT - 8, 1))
            for m in range(MT):
                if xt_ahead:
                    xt = xt_ahead.pop(0)
                else:
                    xt = load_xt(m)

                for _ in range(per_iter):
                    job = next(pending, None)
                    if job is not None:
                        job()

                psums = alloc_psums(m)
                for k in range(KT):
                    lhsT = xt[:, k, :]
                    for p in g:
                        for j in range(2):
                            nc.tensor.matmul(
                                psums[p * 2 + j][:], lhsT,
                                wt[(k, p)][:, j * 512:(j + 1) * 512],
                                start=(k == 0), stop=(k == KT - 1))
                evict(psums, m)
            for job in pending:
                job()

    nc.compile()
    return nc


_PROGRAM_CACHE = {}


def _get_program():
    key = (N_TOK, IN_DIM, OUT_S)
    if key not in _PROGRAM_CACHE:
        _PROGRAM_CACHE[key] = build_program()
    return _PROGRAM_CACHE[key]


def make_in_maps(x, weight_mu, weight_log_sigma, bias_mu, bias_log_sigma,
                 eps_w, eps_b):
    x = np.asarray(x, dtype=np.float32)
    weight_mu = np.asarray(weight_mu, dtype=np.float32)
    weight_log_sigma = np.asarray(weight_log_sigma, dtype=np.float32)
    bias_mu = np.asarray(bias_mu, dtype=np.float32)
    bias_log_sigma = np.asarray(bias_log_sigma, dtype=np.float32)
    eps_w = np.asarray(eps_w, dtype=np.float32)
    eps_b = np.asarray(eps_b, dtype=np.float32)

    # xB[m, ki, ko, t] = x[m*128 + t, ko*128 + ki]
    MT, KT = N_TOK // P, IN_DIM // P
    xB = x.reshape(MT, P, KT, P).transpose(0, 3, 2, 1).astype(ml_dtypes.bfloat16)
    in_maps = []
    for c in range(N_CORES):
        sl = slice(c * OUT_S, (c + 1) * OUT_S)
        in_maps.append({
            "xB": xB,
            "muT": weight_mu[sl].T.astype(np.float16),
            "lsT": weight_log_sigma[sl].T.astype(np.float16),
            "epsT": eps_w[sl].T.astype(np.float16),
            "bmu": np.ascontiguousarray(bias_mu[sl]),
            "bls": np.ascontiguousarray(bias_log_sigma[sl]),
            "beps": np.ascontiguousarray(eps_b[sl]),
        })
    return in_maps


def run(in_maps, trace=False, **kwargs):
    import time
    from concourse.bass_utils import run_bass_kernel_spmd
    nc = _get_program()
    for attempt in range(3):
        try:
            res = run_bass_kernel_spmd(nc, in_maps, list(range(N_CORES)),
                                       trace=trace, **kwargs)
            break
        except Exception:  # transient NRT_EXEC_UNIT_UNRECOVERABLE
            if attempt == 2:
                raise
            time.sleep(15)
    out = np.concatenate([res.results[c]["y"] for c in range(N_CORES)], axis=1)
    return out, res


def kernel(x, weight_mu, weight_log_sigma, bias_mu, bias_log_sigma,
           eps_w, eps_b):
    in_maps = make_in_maps(x, weight_mu, weight_log_sigma, bias_mu,
                           bias_log_sigma, eps_w, eps_b)
    out, _ = run(in_maps, trace=False)
    return out



# revision 3
# speedup vs baseline: 1.2130x; 1.2130x over previous
"""Bayesian linear layer (reparameterized) on 8 Trainium2 NeuronCores.

y = x @ (mu + exp(log_sigma) * eps_w).T + (bias_mu + exp(bias_log_sigma) * eps_b)

Shapes: x [8192, 4096] f32, weights [16384, 4096] f32, y [8192, 16384] f32.

Strategy (column-parallel over out_features, mixed-precision contraction):
  - Shard all [OUT, IN] weight tensors and the bias vectors along OUT across
    8 cores (2048 out features per core); replicate x.
  - The contraction dim (4096) is split: the first KF8*256 rows run as
    fp8e4 DoubleRow matmuls (2 k-rows per PE cell per cycle, ~1.8x bf16
    throughput), the rest as bf16 matmuls, all accumulating into the same
    fp32 PSUM bank. e4m3 quantization of both operands costs ~2.65% rms
    per operand, so the blended rel err is ~0.0375*sqrt(KF8/16) on the y
    norm; KF8 is chosen to stay inside the 2e-2 gate with margin.
  - Scaling: x is pre-scaled by 1/16 on host (exact for bf16; keeps fp8
    subnormal loss negligible) and W by 16 (folded into mu*16 / ls+ln16 on
    host, exact), so PSUM accumulates the UNSCALED product and eviction is
    a single bias add.
  - On device: build W.T = mu16 + exp(ls16)*eps in f32 from fp16 inputs;
    write bf16 tiles for the bf16 k-range and e4m3 DoubleRow-pair tiles
    [128, 2, 1024] for the fp8 k-range. W stays SBUF-resident. Output is
    computed group-by-group (1024 out cols each) so the first group's
    token sweep starts as soon as its W is built while the second group's
    weight inputs stream in behind it. Bias is added during PSUM eviction
    on the vector engine; y goes out in bf16 on the SWDGE queues.
  - Host concatenates the per-core [8192, 2048] outputs and upcasts to f32.
"""

import os
import sys

sys.path.insert(0, "/opt/trn_rl_repo")
os.environ.setdefault("MYCRO_LOCAL_CACHE", "1")

import numpy as np
import ml_dtypes

N_TOK, IN_DIM, OUT_DIM = 8192, 4096, 16384
N_CORES = 8
OUT_S = OUT_DIM // N_CORES  # 2048
P = 128
KF8 = 4          # fp8 DoubleRow super-tiles (256 k-rows each) out of IN/256
SCALE = 16.0     # x/16 on host, W*16 on host+device; psum is unscaled


def build_program(n_tok=N_TOK, in_dim=IN_DIM, out_s=OUT_S, n_cores=N_CORES,
                  kf8=KF8, xt_bufs=3, out_bufs=4, psum_bufs=8):
    """Build + compile the single-core Bass program (SPMD across cores)."""
    import concourse.bass as bass
    import concourse.mybir as mybir
    import concourse.tile as tile
    from concourse import bacc
    from contextlib import ExitStack

    fp32 = mybir.dt.float32
    bf16 = mybir.dt.bfloat16
    fp16 = mybir.dt.float16
    fp8 = mybir.dt.float8e4
    Exp = mybir.ActivationFunctionType.Exp
    add = mybir.AluOpType.add
    DR = mybir.MatmulPerfMode.DoubleRow

    KF = kf8 * 256                  # fp8 contraction rows
    KB = (in_dim - KF) // P         # bf16 k tiles
    MT = n_tok // P                 # token tiles
    GW = 1024                       # out-column group width
    NG = out_s // GW                # groups
    assert in_dim % P == 0 and n_tok % P == 0 and out_s % GW == 0
    assert KF <= in_dim and (in_dim - KF) % P == 0

    nc = bacc.Bacc("TRN2", target_bir_lowering=False, debug=False,
                   num_devices=n_cores, enable_asserts=False)

    # x pre-tiled on host (values x/16):
    #   xB8[m, p, ks, j, t] = xs[m*128 + t, ks*256 + j*128 + p]   (e4m3)
    #   xB16[m, ki, kb, t]  = xs[m*128 + t, KF + kb*128 + ki]     (bf16)
    if kf8:
        xB8 = nc.dram_tensor("xB8", [MT, P, kf8, 2, P], fp8,
                             kind="ExternalInput")
    if KB:
        xB16 = nc.dram_tensor("xB16", [MT, P, KB, P], bf16,
                              kind="ExternalInput")
    # fp16 staging of the W-build inputs (host folds the *16 / +ln16 scale
    # in); fp16's 10 mantissa bits keep the build at f32-input accuracy.
    muT = nc.dram_tensor("muT", [in_dim, out_s], fp16, kind="ExternalInput")
    lsT = nc.dram_tensor("lsT", [in_dim, out_s], fp16, kind="ExternalInput")
    epsT = nc.dram_tensor("epsT", [in_dim, out_s], fp16, kind="ExternalInput")
    bmu = nc.dram_tensor("bmu", [out_s], fp32, kind="ExternalInput")
    bls = nc.dram_tensor("bls", [out_s], fp32, kind="ExternalInput")
    beps = nc.dram_tensor("beps", [out_s], fp32, kind="ExternalInput")
    y = nc.dram_tensor("y", [n_tok, out_s], bf16, kind="ExternalOutput")

    with tile.TileContext(nc) as tc, ExitStack() as ctx:
        wt_pool = ctx.enter_context(tc.tile_pool(name="wt", bufs=1))
        const_pool = ctx.enter_context(tc.tile_pool(name="const", bufs=1))
        scratch = ctx.enter_context(tc.tile_pool(name="scratch", bufs=3))
        xt_pool = ctx.enter_context(tc.tile_pool(name="xt", bufs=xt_bufs))
        out_pool = ctx.enter_context(tc.tile_pool(name="out", bufs=out_bufs))
        psum_pool = ctx.enter_context(
            tc.tile_pool(name="psum", bufs=psum_bufs, space="PSUM"))

        def fused_w(dst_ap, ls_src, eps_src, mu_src, sync_engine,
                    dt_ls, dt_em, pre="", bufs=None, width=GW):
            # dst = mu + exp(ls) * eps, elementwise over a [P, width] block
            kw = {} if bufs is None else {"bufs": bufs}
            l = scratch.tile([P, width], dt_ls, tag=pre + "ls", name="ls_t", **kw)
            e = scratch.tile([P, width], dt_em, tag=pre + "eps", name="eps_t", **kw)
            m_ = scratch.tile([P, width], dt_em, tag=pre + "mu", name="mu_t", **kw)
            x_ = scratch.tile([P, width], fp32, tag=pre + "exp",
                              name="exp_t", **kw)
            sync_engine.dma_start(out=l[:], in_=ls_src)
            sync_engine.dma_start(out=e[:], in_=eps_src)
            sync_engine.dma_start(out=m_[:], in_=mu_src)
            nc.scalar.activation(x_[:], l[:], Exp)
            nc.vector.tensor_mul(x_[:], x_[:], e[:])
            nc.vector.tensor_tensor(dst_ap, x_[:], m_[:], add)

        # bias_rep[p, o] = bmu[o] + exp(bls[o]) * beps[o] (unscaled; added
        # into the f32 psum at eviction)
        bias_rep = const_pool.tile([P, out_s], bf16, tag="bias_rep",
                                   name="bias_rep")

        def bias_chunk(j):
            sl = slice(j * 512, (j + 1) * 512)
            fused_w(bias_rep[:, sl],
                    bls.ap()[sl].partition_broadcast(P),
                    beps.ap()[sl].partition_broadcast(P),
                    bmu.ap()[sl].partition_broadcast(P),
                    nc.gpsimd, fp32, fp32, pre="b", bufs=1, width=512)

        # ---- W tiles ----
        # fp8 DoubleRow pair tiles: w8[(ks, g)] is [P, 2, GW] e4m3 holding
        # contraction rows ks*256 + j*128 + p for out cols [g*GW, (g+1)*GW).
        # bf16 tiles: w16[(kb, g)] is [P, GW] for rows KF + kb*128 + p.
        w8 = {}
        w16 = {}

        def build_w8_chunk(ks, g, j):
            key = (ks, g)
            if key not in w8:
                w8[key] = wt_pool.tile([P, 2, GW], fp8, tag=f"w8_{ks}_{g}",
                                       name=f"w8_{ks}_{g}")
            rows = slice(ks * 256 + j * P, ks * 256 + (j + 1) * P)
            cols = slice(g * GW, (g + 1) * GW)
            fused_w(w8[key][:, j, :],
                    lsT.ap()[rows, cols],
                    epsT.ap()[rows, cols],
                    muT.ap()[rows, cols],
                    nc.sync, fp16, fp16, bufs=2)

        def build_w16_chunk(kb, g):
            t = wt_pool.tile([P, GW], bf16, tag=f"w16_{kb}_{g}",
                             name=f"w16_{kb}_{g}")
            w16[(kb, g)] = t
            rows = slice(KF + kb * P, KF + (kb + 1) * P)
            cols = slice(g * GW, (g + 1) * GW)
            fused_w(t[:],
                    lsT.ap()[rows, cols],
                    epsT.ap()[rows, cols],
                    muT.ap()[rows, cols],
                    nc.sync, fp16, fp16, bufs=2)

        def group_jobs(g):
            return ([(lambda ks=ks, j=j: build_w8_chunk(ks, g, j))
                     for ks in range(kf8) for j in range(2)]
                    + [(lambda kb=kb: build_w16_chunk(kb, g))
                       for kb in range(KB)])

        def load_xt(m):
            ts = []
            if kf8:
                t8 = xt_pool.tile([P, kf8, 2, P], fp8, tag="xt8", name="xt8")
                nc.sync.dma_start(out=t8[:], in_=xB8.ap()[m])
                ts.append(t8)
            else:
                ts.append(None)
            if KB:
                t16 = xt_pool.tile([P, KB, P], bf16, tag="xt16", name="xt16")
                nc.sync.dma_start(out=t16[:], in_=xB16.ap()[m])
                ts.append(t16)
            else:
                ts.append(None)
            return ts

        xt_ahead = []  # prefetched token tiles, consumed by the first iters

        for g in range(NG):
            if g == 0:
                xt_ahead.append(load_xt(0))
                # Warm-up: throwaway matmuls with no W dependency keep the
                # PE dense through the W-build window, so the HAM clock gate
                # opens to 8/8 once and stays (idle >3.4us re-throttles to
                # half clock).
                xt8_0, xt16_0 = xt_ahead[0]
                warm_ps = psum_pool.tile([P, 512], fp32, tag="ps",
                                         name="warm_ps")
                if KB >= 2:
                    wa, wb = xt16_0[:, 0, :], xt16_0[:, 1, :]
                else:
                    wa = wb = xt8_0[:, 0, 0, :]
                for _ in range(60):
                    nc.tensor.matmul(warm_ps[:, :P], wa, wb,
                                     start=True, stop=True)
                while len(xt_ahead) < min(xt_bufs, MT):
                    xt_ahead.append(load_xt(len(xt_ahead)))
                for job in group_jobs(0):
                    job()
                for j in range(out_s // 512):
                    bias_chunk(j)
            # Next group's W-build chunks are interleaved into this group's
            # m-loop below so their DMA/ACT/DVE work overlaps matmul compute
            # instead of queueing behind the whole group in program order.
            pending = group_jobs(g + 1) if g + 1 < NG else []
            n_pending = len(pending)
            pending = iter(pending)
            ocs = [g * (GW // 512) + j for j in range(GW // 512)]

            def evict(psums, m):
                for oc in ocs:
                    ot = out_pool.tile([P, 512], bf16, tag="ot", name="ot")
                    nc.vector.tensor_tensor(ot[:], psums[oc][:],
                                            bias_rep[:, oc * 512:(oc + 1) * 512],
                                            add)
                    # SWDGE (gpsimd): y stores wait on the eviction, and on
                    # the sync stream that wait would head-of-line-block the
                    # next x-tile load; stores are latency-insensitive, so
                    # keep them off the load queues entirely.
                    nc.gpsimd.dma_start(
                        out=y.ap()[m * P:(m + 1) * P, oc * 512:(oc + 1) * 512],
                        in_=ot[:])

            per_iter = -(-n_pending // max(MT - 8, 1))
            for m in range(MT):
                if xt_ahead:
                    xt8_t, xt16_t = xt_ahead.pop(0)
                else:
                    xt8_t, xt16_t = load_xt(m)

                for _ in range(per_iter):
                    job = next(pending, None)
                    if job is not None:
                        job()

                psums = {oc: psum_pool.tile([P, 512], fp32, tag="ps",
                                            name=f"ps{m}_{oc}")
                         for oc in ocs}
                for ks in range(kf8):
                    lhsT = xt8_t[:, ks, :, :]
                    for i, oc in enumerate(ocs):
                        nc.tensor.matmul(
                            psums[oc][:], lhsT,
                            w8[(ks, g)][:, :, i * 512:(i + 1) * 512],
                            start=(ks == 0), stop=(KB == 0 and ks == kf8 - 1),
                            perf_mode=DR)
                for kb in range(KB):
                    lhsT = xt16_t[:, kb, :]
                    for i, oc in enumerate(ocs):
                        nc.tensor.matmul(
                            psums[oc][:], lhsT,
                            w16[(kb, g)][:, i * 512:(i + 1) * 512],
                            start=(kf8 == 0 and kb == 0), stop=(kb == KB - 1))
                evict(psums, m)
            for job in pending:
                job()

    nc.compile()
    return nc


_PROGRAM_CACHE = {}


def _get_program():
    key = (N_TOK, IN_DIM, OUT_S, KF8)
    if key not in _PROGRAM_CACHE:
        _PROGRAM_CACHE[key] = build_program()
    return _PROGRAM_CACHE[key]


def make_in_maps(x, weight_mu, weight_log_sigma, bias_mu, bias_log_sigma,
                 eps_w, eps_b, kf8=KF8):
    x = np.asarray(x, dtype=np.float32)
    weight_mu = np.asarray(weight_mu, dtype=np.float32)
    weight_log_sigma = np.asarray(weight_log_sigma, dtype=np.float32)
    bias_mu = np.asarray(bias_mu, dtype=np.float32)
    bias_log_sigma = np.asarray(bias_log_sigma, dtype=np.float32)
    eps_w = np.asarray(eps_w, dtype=np.float32)
    eps_b = np.asarray(eps_b, dtype=np.float32)

    KF = kf8 * 256
    MT, KB = N_TOK // P, (IN_DIM - KF) // P
    xs = x * np.float32(1.0 / SCALE)
    # xB8[m, p, ks, j, t] = xs[m*128 + t, ks*256 + j*128 + p]
    xB8 = np.ascontiguousarray(
        xs[:, :KF].reshape(MT, P, kf8, 2, P).transpose(0, 4, 2, 3, 1)
    ).astype(ml_dtypes.float8_e4m3)
    # xB16[m, ki, kb, t] = xs[m*128 + t, KF + kb*128 + ki]
    xB16 = np.ascontiguousarray(
        xs[:, KF:].reshape(MT, P, KB, P).transpose(0, 3, 2, 1)
    ).astype(ml_dtypes.bfloat16)
    LN = np.float32(np.log(SCALE))
    in_maps = []
    for c in range(N_CORES):
        sl = slice(c * OUT_S, (c + 1) * OUT_S)
        im = {
            "muT": (weight_mu[sl].T * np.float32(SCALE)).astype(np.float16),
            "lsT": (weight_log_sigma[sl].T + LN).astype(np.float16),
            "epsT": eps_w[sl].T.astype(np.float16),
            "bmu": np.ascontiguousarray(bias_mu[sl]),
            "bls": np.ascontiguousarray(bias_log_sigma[sl]),
            "beps": np.ascontiguousarray(eps_b[sl]),
        }
        if KF:
            im["xB8"] = xB8
        if KB:
            im["xB16"] = xB16
        in_maps.append(im)
    return in_maps


def run(in_maps, trace=False, **kwargs):
    import time
    from concourse.bass_utils import run_bass_kernel_spmd
    nc = _get_program()
    for attempt in range(3):
        try:
            res = run_bass_kernel_spmd(nc, in_maps, list(range(N_CORES)),
                                       trace=trace, **kwargs)
            break
        except Exception:  # transient NRT_EXEC_UNIT_UNRECOVERABLE
            if attempt == 2:
                raise
            time.sleep(15)
    out = np.concatenate(
        [np.asarray(res.results[c]["y"]).astype(np.float32)
         for c in range(N_CORES)], axis=1)
    return out, res


def kernel(x, weight_mu, weight_log_sigma, bias_mu, bias_log_sigma,
           eps_w, eps_b):
    in_maps = make_in_maps(x, weight_mu, weight_log_sigma, bias_mu,
                           bias_log_sigma, eps_w, eps_b)
    out, _ = run(in_maps, trace=False)
    return out
